# Initial kernel scaffold
#
"""Fused MergedQKVParallelLinearWithDelta kernel for 8 Trainium2 NeuronCores.

Strategy (tensor-parallel on the QKV output dim, as in vLLM):
  - Each core owns a 768-row output shard (512 q + 128 k + 128 v rows).
  - Tokens are sorted by adapter index on the host; the device gathers token
    rows with an indirect DMA (f32->f16 cast in flight), permutes the K dim to
    nibble-extraction order, and DMA-transposes to K-major tiles.
  - GPTQ 4-bit delta weights are DMA-transposed as uint16, nibble-extracted on
    DVE (fused shift+and), and converted to f16 (DVE/GPSIMD).
  - All matmuls compute out^T [o, t] with weights stationary, so scales/zeros
    are per-partition scalars: out = psum_base + sc[o] * psum_delta where
    psum_delta = sum_k x*w4 - (z+1)*rowsum(x) (the z term is a rank-1 matmul).
  - The host de-permutes/reassembles the 8 transposed shards.
"""

import math
from contextlib import ExitStack

import numpy as np

import concourse.bass as bass
import concourse.tile as tile
from concourse import bacc
from concourse import mybir
from concourse.bass_utils import run_bass_kernel_spmd

N_CORES = 8
T, IN = 1024, 4096
Q, KV = 4096, 1024
OUT = Q + 2 * KV
D = 4
OS = OUT // N_CORES          # 768 output rows per core
NB = IN // 128               # 32 K' tiles
SLICE = 512
PACKW = IN // 4              # 1024 uint16 words per row

F16 = mybir.dt.float16
F32 = mybir.dt.float32
U16 = mybir.dt.uint16
I32 = mybir.dt.int32

# ---------------------------------------------------------------------------
# Host-side routing schedule
# ---------------------------------------------------------------------------
def _schedule(indices):
    idx = np.asarray(indices).astype(np.int64)
    tile_adapters = []
    gather_parts = []
    orig_parts = []
    for d in range(D):
        toks = np.nonzero(idx == d)[0]
        if len(toks) == 0:
            continue
        n_t = (len(toks) + 127) // 128
        pad = n_t * 128 - len(toks)
        gather_parts.append(np.concatenate([toks, np.zeros(pad, np.int64)]))
        orig_parts.append(np.concatenate([toks, -np.ones(pad, np.int64)]))
        tile_adapters += [d] * n_t
    gather = np.concatenate(gather_parts).astype(np.int32)
    origs = np.concatenate(orig_parts).astype(np.int64)
    return tuple(tile_adapters), gather, origs


def _slices_and_runs(tile_adapters):
    n_tiles = len(tile_adapters)
    t_pad = n_tiles * 128
    slices = []
    c = 0
    while c < t_pad:
        slices.append((c, min(c + SLICE, t_pad)))
        c += SLICE
    runs = []  # per slice: list of (col0_in_slice, ncols, adapter)
    for c0, c1 in slices:
        rr = []
        for i in range(c0 // 128, c1 // 128):
            d = tile_adapters[i]
            col = i * 128 - c0
            if rr and rr[-1][2] == d and rr[-1][0] + rr[-1][1] == col:
                rr[-1] = (rr[-1][0], rr[-1][1] + 128, d)
            else:
                rr.append((col, 128, d))
        runs.append(rr)
    return slices, runs


# ---------------------------------------------------------------------------
# Device program
# ---------------------------------------------------------------------------
DEBUG_TAPS = 0


def _build_program(tile_adapters, split_waits=True):
    n_tiles = len(tile_adapters)
    t_pad = n_tiles * 128
    slices, runs = _slices_and_runs(tile_adapters)
    n_s = len(slices)
    adapters_present = sorted(set(tile_adapters))

    nc = bacc.Bacc(
        trn_type="TRN2", target_bir_lowering=False, debug=False, num_devices=1
    )
    x_d = nc.dram_tensor("x", [T, IN], F32, kind="ExternalInput").ap()
    gidx_d = nc.dram_tensor("gidx", [t_pad, 1], I32, kind="ExternalInput").ap()
    wb_d = nc.dram_tensor("wb", [OS, IN], F32, kind="ExternalInput").ap()
    qwu_d = nc.dram_tensor("qwu", [D, OS, PACKW], U16, kind="ExternalInput").ap()
    biasr_d = nc.dram_tensor("biasr", [1, OS], F16, kind="ExternalInput").ap()
    znr_d = nc.dram_tensor("znr", [1, D * OS], F16, kind="ExternalInput").ap()
    scc_d = nc.dram_tensor("scc", [128, (OS // 128) * D], F32, kind="ExternalInput").ap()
    outT_d = nc.dram_tensor("outT", [OS, t_pad], F32, kind="ExternalOutput").ap()
    if DEBUG_TAPS in (2, 3, 4):
        wdump_d = nc.dram_tensor(
            "wdump", [OS // 128, NB, 128, 128 * (1 + D)], F16, kind="ExternalOutput"
        ).ap()
    if DEBUG_TAPS in (1, 3):
        xdump_d = nc.dram_tensor(
            "xdump", [NB, 128, t_pad], F16, kind="ExternalOutput"
        ).ap()
        udump_d = nc.dram_tensor(
            "udump", [2, t_pad], F16, kind="ExternalOutput"
        ).ap()

    with TileCtx(nc) as tc, ExitStack() as ctx:
        pmisc = ctx.enter_context(tc.tile_pool(name="misc", bufs=1))
        pgi = ctx.enter_context(tc.tile_pool(name="gi", bufs=2))
        pin = ctx.enter_context(tc.tile_pool(name="ain", bufs=2))
        pperm = ctx.enter_context(tc.tile_pool(name="perm", bufs=2))
        pxgT = ctx.enter_context(tc.tile_pool(name="xgT", bufs=1))
        pw = ctx.enter_context(tc.tile_pool(name="wpool", bufs=36))
        pqt = ctx.enter_context(tc.tile_pool(name="qt", bufs=6))
        pext = ctx.enter_context(tc.tile_pool(name="ext", bufs=6))
        pps = ctx.enter_context(tc.tile_pool(name="ps", bufs=1, space="PSUM"))
        pout = ctx.enter_context(tc.tile_pool(name="outp", bufs=4))

        # constants
        biasr = pmisc.tile([1, OS], F16, tag="biasr")
        nc.gpsimd.dma_start(biasr[:], biasr_d[:])
        znr = pmisc.tile([1, D * OS], F16, tag="znr")
        nc.gpsimd.dma_start(znr[:], znr_d[:])
        scc = pmisc.tile([128, (OS // 128) * D], F32, tag="scc")
        nc.gpsimd.dma_start(scc[:], scc_d[:])
        ones_col = pmisc.tile([128, 1], F16, tag="onesc")
        nc.vector.memset(ones_col[:], 1.0)
        ones_row = pmisc.tile([1, SLICE], F16, tag="onesr")
        nc.vector.memset(ones_row[:], 1.0)

        # xgT[kb][s] : [128, slen] f16  (K'-major gathered activations)
        xgT = [
            [
                pxgT.tile(
                    [128, c1 - c0], F16, tag=f"xgT_{kb}_{s}", name=f"xgT_{kb}_{s}"
                )
                for s, (c0, c1) in enumerate(slices)
            ]
            for kb in range(NB)
        ]

        def sigma_copy(dst, src):
            # dst[.., 512*C + 128*jj + p] = src[.., 512*C + 4*p + jj]
            sv = src.rearrange("a (b p j) -> a b j p", b=IN // 512, p=128, j=4)
            dv = dst.rearrange("a (b j p) -> a b j p", b=IN // 512, j=4, p=128)
            nc.scalar.copy(dv, sv)

        # ---- Phase A: gather + permute + transpose activations
        for i in range(n_tiles):
            s_i = (i * 128) // SLICE
            col = i * 128 - slices[s_i][0]
            gi = pgi.tile([128, 1], I32, tag="gi")
            nc.gpsimd.dma_start(gi[:], gidx_d[i * 128 : (i + 1) * 128, :])
            gx = pin.tile([128, IN], F16, tag="ain")
            nc.gpsimd.indirect_dma_start(
                out=gx[:],
                out_offset=None,
                in_=x_d[:],
                in_offset=bass.IndirectOffsetOnAxis(ap=gi[:, :1], axis=0),
            )
            px = pperm.tile([128, IN], F16, tag="perm")
            sigma_copy(px[:], gx[:])
            for kb in range(NB):
                nc.sync.dma_start(
                    xgT[kb][s_i][:, col : col + 128],
                    px[:, kb * 128 : (kb + 1) * 128],
                    transpose=True,
                )

        # ---- u rows: colsum of xg (fp16) per slice
        u_rows = []
        for s, (c0, c1) in enumerate(slices):
            slen = c1 - c0
            ups = pps.tile([1, slen], F32, space="PSUM", tag="ups")
            for kb in range(NB):
                nc.tensor.matmul(
                    ups[:],
                    lhsT=ones_col[:],
                    rhs=xgT[kb][s][:],
                    start=(kb == 0),
                    stop=(kb == NB - 1),
                )
            ur = pmisc.tile([1, slen], F16, tag=f"urow{s}")
            nc.vector.tensor_copy(ur[:], ups[:])
            url = pmisc.tile([1, slen], F16, tag=f"urowl{s}")
            nc.vector.tensor_tensor(
                out=url[:], in0=ups[:], in1=ur[:], op=mybir.AluOpType.subtract
            )
            u_rows.append((ur, url))
            if DEBUG_TAPS in (1, 3):
                nc.sync.dma_start(udump_d[0:1, c0:c1], ur[:])
                nc.sync.dma_start(udump_d[1:2, c0:c1], url[:])

        # ---- Phase B: per 128-row output tile
        conv_rr = 0
        for ot in range(OS // 128):
            o0 = 128 * ot
            orng = slice(o0, o0 + 128)

            # base weights: cast-load, sigma-permute, transpose into Wt[:, 0:128]
            wbt = pin.tile([128, IN], F16, tag="ain")
            nc.gpsimd.dma_start(wbt[:], wb_d[orng, :])
            wbp = pperm.tile([128, IN], F16, tag="perm")
            sigma_copy(wbp[:], wbt[:])
            wt = [pw.tile([128, 128 * (1 + D)], F16, tag="W", name=f"wt_{ot}_{k}") for k in range(NB)]
            for kb in range(NB):
                nc.sync.dma_start(
                    wt[kb][:, 0:128],
                    wbp[:, kb * 128 : (kb + 1) * 128],
                    transpose=True,
                )

            # delta weights: u16 transpose + nibble extract + convert
            for C in range(PACKW // 128):
                qt = pqt.tile([128, 128 * D], U16, tag="qt")
                for d in range(D):
                    nc.sync.dma_start(
                        qt[:, d * 128 : (d + 1) * 128],
                        qwu_d[d, orng, C * 128 : (C + 1) * 128],
                        transpose=True,
                    )
                for jj in range(4):
                    kb = 4 * C + jj
                    ex = pext.tile([128, 128 * D], U16, tag="ex")
                    if jj == 0:
                        nc.vector.tensor_scalar(
                            out=ex[:], in0=qt[:], scalar1=0xF, scalar2=None,
                            op0=mybir.AluOpType.bitwise_and,
                        )
                    elif jj == 3:
                        nc.vector.tensor_scalar(
                            out=ex[:], in0=qt[:], scalar1=12, scalar2=None,
                            op0=mybir.AluOpType.logical_shift_right,
                        )
                    else:
                        nc.vector.tensor_scalar(
                            out=ex[:], in0=qt[:], scalar1=4 * jj, scalar2=0xF,
                            op0=mybir.AluOpType.logical_shift_right,
                            op1=mybir.AluOpType.bitwise_and,
                        )
                    eng = nc.vector  # gpsimd convert suspected racy
                    eng.tensor_copy(wt[kb][:, 128 : 128 * (1 + D)], ex[:])
                    conv_rr += 1

            if DEBUG_TAPS in (2, 3):
                for kb in range(NB):
                    nc.sync.dma_start(wdump_d[ot, kb], wt[kb][:])
            if DEBUG_TAPS in (1, 3) and ot == 0:
                for kb in range(NB):
                    for s_, (c0_, c1_) in enumerate(slices):
                        nc.sync.dma_start(
                            xdump_d[kb, :, c0_:c1_], xgT[kb][s_][:]
                        )
            # matmuls: out^T accumulation
            psb = []
            psd = []
            for s, (c0, c1) in enumerate(slices):
                slen = c1 - c0
                b = pps.tile([128, slen], F32, space="PSUM", tag=f"psb{s}")
                dl = pps.tile([128, slen], F32, space="PSUM", tag=f"psd{s}")
                psb.append(b)
                psd.append(dl)
                nc.tensor.matmul(
                    b[:],
                    lhsT=biasr[0:1, orng],
                    rhs=ones_row[0:1, 0:slen],
                    start=True,
                    stop=False,
                )
            for kb in range(NB):
                for s in range(n_s):
                    nc.tensor.matmul(
                        psb[s][:],
                        lhsT=wt[kb][:, 0:128],
                        rhs=xgT[kb][s][:],
                        start=False,
                        stop=(kb == NB - 1),
                    )
                    for ri, (rc0, rn, d) in enumerate(runs[s]):
                        nc.tensor.matmul(
                            psd[s][:, rc0 : rc0 + rn],
                            lhsT=wt[kb][:, 128 * (1 + d) : 128 * (2 + d)],
                            rhs=xgT[kb][s][:, rc0 : rc0 + rn],
                            start=(kb == 0 and ri == 0),
                            stop=False,
                        )
            for s in range(n_s):
                for ri, (rc0, rn, d) in enumerate(runs[s]):
                    for ui, upart in enumerate(u_rows[s]):
                        nc.tensor.matmul(
                            psd[s][:, rc0 : rc0 + rn],
                            lhsT=znr[0:1, d * OS + o0 : d * OS + o0 + 128],
                            rhs=upart[0:1, rc0 : rc0 + rn],
                            start=False,
                            stop=(ri == len(runs[s]) - 1 and ui == 1),
                        )
                oo = pout.tile([128, slices[s][1] - slices[s][0]], F32, tag="o")
                tmp = pout.tile(
                    [128, slices[s][1] - slices[s][0]], F32, tag="otmp"
                )
                for rc0, rn, d in runs[s]:
                    nc.scalar.mul(
                        tmp[:, rc0 : rc0 + rn],
                        psd[s][:, rc0 : rc0 + rn],
                        scc[:, ot * D + d : ot * D + d + 1],
                    )
                nc.vector.tensor_tensor(
                    out=oo[:],
                    in0=tmp[:],
                    in1=psb[s][:],
                    op=mybir.AluOpType.add,
                )
                nc.gpsimd.dma_start(
                    outT_d[orng, slices[s][0] : slices[s][1]], oo[:]
                )

    if split_waits:
        nc.compile()
    return nc


def TileCtx(nc):
    return tile.TileContext(nc)


# ---------------------------------------------------------------------------
# Host wrapper
# ---------------------------------------------------------------------------
def _unpack_zeros(qz, o_count):
    # qz: [D, o_count//8, 1] int32; returns [D, o_count] float zeros
    o = np.arange(o_count)
    words = qz[:, o >> 3, 0].astype(np.int64)
    return ((words >> (4 * (o & 7))) & 0xF).astype(np.float32)


_prog_cache = {}


def kernel(**inputs):
    x = np.ascontiguousarray(np.asarray(inputs["x"], dtype=np.float32))
    w_base = np.asarray(inputs["w_base"], dtype=np.float32)
    bias = np.asarray(inputs["bias"], dtype=np.float32)
    qw_q = np.asarray(inputs["qweight_q"], dtype=np.int32)
    qw_k = np.asarray(inputs["qweight_k"], dtype=np.int32)
    qw_v = np.asarray(inputs["qweight_v"], dtype=np.int32)
    qz_q = np.asarray(inputs["qzeros_q"], dtype=np.int32)
    qz_k = np.asarray(inputs["qzeros_k"], dtype=np.int32)
    qz_v = np.asarray(inputs["qzeros_v"], dtype=np.int32)
    sc_q = np.asarray(inputs["scales_q"], dtype=np.float32)
    sc_k = np.asarray(inputs["scales_k"], dtype=np.float32)
    sc_v = np.asarray(inputs["scales_v"], dtype=np.float32)
    indices = np.asarray(inputs["indices"])

    tile_adapters, gather, origs = _schedule(indices)
    t_pad = len(tile_adapters) * 128

    if tile_adapters not in _prog_cache:
        _prog_cache[tile_adapters] = _build_program(tile_adapters)
    nc = _prog_cache[tile_adapters]

    z_q = _unpack_zeros(qz_q, Q)
    z_k = _unpack_zeros(qz_k, KV)
    z_v = _unpack_zeros(qz_v, KV)

    SQ, SK = Q // N_CORES, KV // N_CORES
    in_maps = []
    for c in range(N_CORES):
        qs = slice(SQ * c, SQ * (c + 1))
        ks = slice(SK * c, SK * (c + 1))
        wb = np.concatenate(
            [w_base[qs], w_base[Q + SK * c : Q + SK * (c + 1)],
             w_base[Q + KV + SK * c : Q + KV + SK * (c + 1)]], axis=0
        )
        qw = np.concatenate([qw_q[:, qs], qw_k[:, ks], qw_v[:, ks]], axis=1)
        qwu = np.ascontiguousarray(qw).view(np.uint16).reshape(D, OS, PACKW)
        z = np.concatenate([z_q[:, qs], z_k[:, ks], z_v[:, ks]], axis=1)
        sc = np.concatenate(
            [sc_q[:, qs, 0], sc_k[:, ks, 0], sc_v[:, ks, 0]], axis=1
        )
        b = np.concatenate(
            [bias[qs], bias[Q + SK * c : Q + SK * (c + 1)],
             bias[Q + KV + SK * c : Q + KV + SK * (c + 1)]]
        )
        znr = (-(z + 1.0)).astype(np.float16)
        biasr = np.ascontiguousarray(b.astype(np.float16)[None, :])
        scc = np.zeros([128, (OS // 128) * D], np.float32)
        for ot in range(OS // 128):
            for d in range(D):
                scc[:, ot * D + d] = sc[d, 128 * ot : 128 * (ot + 1)]
        in_maps.append(
            {
                "x": x,
                "gidx": np.ascontiguousarray(gather[:, None]),
                "wb": np.ascontiguousarray(wb),
                "qwu": qwu,
                "biasr": biasr,
                "znr": np.ascontiguousarray(znr.reshape(1, -1)),
                "scc": scc,
            }
        )

    import os

    trace = bool(int(os.environ.get("KERNEL_TRACE", "0")))
    res = run_bass_kernel_spmd(
        nc, in_maps, core_ids=list(range(N_CORES)), trace=trace
    )
    kernel._last_results = res

    out = np.zeros([T, OUT], np.float32)
    valid = origs >= 0
    vpos = np.nonzero(valid)[0]
    vtok = origs[valid]
    for c in range(N_CORES):
        rT = res.results[c]["outT"]  # [OS, t_pad]
        r = np.asarray(rT).T  # [t_pad, OS]
        cols = np.concatenate(
            [
                np.arange(SQ * c, SQ * (c + 1)),
                np.arange(Q + SK * c, Q + SK * (c + 1)),
                np.arange(Q + KV + SK * c, Q + KV + SK * (c + 1)),
            ]
        )
        out[vtok[:, None], cols[None, :]] = r[vpos]
    return out



# revision 28
# speedup vs baseline: 5.3485x; 5.3485x over previous
"""Fused MergedQKVParallelLinearWithDelta kernel for 8 Trainium2 NeuronCores.

Strategy (tensor-parallel on the QKV output dim, as in vLLM):
  - Each core owns a 768-row output shard (512 q + 128 k + 128 v rows).
  - All layout work happens host-side: tokens are sorted by adapter and
    x is pre-transposed to k-major f16 tiles; w_base shards are
    pre-transposed/cast the same way; the packed GPTQ words are
    pre-transposed to k-major uint16 so the device only does dense 2D
    DMA loads (no gathers, no DMA transposes).
  - Device work per 128-token tile: 32 k-tiles x 4 matmuls (base 512+256,
    delta 512+256 cols) with the x tile stationary, then one DVE
    combine out = psum_base + scale * psum_delta and a dense DMA out.
  - GPTQ nibble extraction (u16 shift/and -> f16) runs on DVE, one
    adapter stack (6.3 MB) at a time, double-buffered.
  - The rank-1 terms (bias, -(z+1)*scale * rowsum(x)) are added on the
    host, exactly, after the device returns the matmul parts.
"""

from contextlib import ExitStack

import numpy as np

import concourse.bass as bass  # noqa: F401  (kept for parity with bass_utils)
import concourse.tile as tile
from concourse import bacc
from concourse import mybir
from concourse.bass_utils import run_bass_kernel_spmd

N_CORES = 8
T, IN = 1024, 4096
Q, KV = 4096, 1024
OUT = Q + 2 * KV
D = 4
OS = OUT // N_CORES          # 768 output rows per core
NB = IN // 128               # 32 k-tiles

F16 = mybir.dt.float16
F32 = mybir.dt.float32
U16 = mybir.dt.uint16

# k-permutation: k-tile kb, partition p holds original k = 512*(kb//4) + 4*p + (kb%4)
# (nibble j of u16 word pw covers k = 4*pw + j; block b = pw//128, p = pw%128, kb = 4*b + j)
_kb = np.arange(NB)[:, None]
_p = np.arange(128)[None, :]
KIDX = (512 * (_kb // 4) + 4 * _p + (_kb % 4)).reshape(-1)  # [NB*128]


# ---------------------------------------------------------------------------
# Host-side routing schedule: tokens sorted by adapter, groups padded to 64.
# A 128-token tile is two 64-blocks; a tile whose blocks belong to different
# adapters runs its delta as two M=64 col-tiled matmuls (concurrent on PE).
# ---------------------------------------------------------------------------
BLK = 64


def _schedule(indices):
    idx = np.asarray(indices).astype(np.int64)
    gather_parts = []
    orig_parts = []
    seg_adapters = []
    for d in range(D):
        toks = np.nonzero(idx == d)[0]
        if len(toks) == 0:
            continue
        n_b = (len(toks) + BLK - 1) // BLK
        pad = n_b * BLK - len(toks)
        gather_parts.append(np.concatenate([toks, np.zeros(pad, np.int64)]))
        orig_parts.append(np.concatenate([toks, -np.ones(pad, np.int64)]))
        seg_adapters += [d] * n_b
    gather = np.concatenate(gather_parts)
    origs = np.concatenate(orig_parts)
    if len(seg_adapters) % 2:
        seg_adapters.append(seg_adapters[-1])
        gather = np.concatenate([gather, np.zeros(BLK, np.int64)])
        origs = np.concatenate([origs, -np.ones(BLK, np.int64)])
    n_tiles = len(seg_adapters) // 2
    tiles = []
    for ti in range(n_tiles):
        d0, d1 = seg_adapters[2 * ti], seg_adapters[2 * ti + 1]
        if d0 == d1:
            tiles.append(((0, 128, d0),))
        else:
            tiles.append(((0, 64, d0), (64, 64, d1)))
    return tuple(tiles), gather, origs


# ---------------------------------------------------------------------------
# Device program
# ---------------------------------------------------------------------------
def _build_program(tiles_spec):
    n_tiles = len(tiles_spec)

    nc = bacc.Bacc(
        trn_type="TRN2", target_bir_lowering=False, debug=False, num_devices=1
    )
    xst_d = nc.dram_tensor("xst", [n_tiles, 128, NB * 128], F16, kind="ExternalInput").ap()
    wbt_d = nc.dram_tensor("wbt", [128, NB * OS], F16, kind="ExternalInput").ap()
    qst_d = nc.dram_tensor("qst", [D, 8, 128, OS], U16, kind="ExternalInput").ap()
    scb_d = nc.dram_tensor("scb", [D, 128, OS], F32, kind="ExternalInput").ap()
    out_d = nc.dram_tensor("outD", [n_tiles, 128, OS], F32, kind="ExternalOutput").ap()
    warm_d = nc.dram_tensor("warmD", [1, 1], F32, kind="ExternalOutput").ap()

    with tile.TileContext(nc) as tc, ExitStack() as ctx:
        pwb = ctx.enter_context(tc.tile_pool(name="wb", bufs=1))
        pwd = ctx.enter_context(tc.tile_pool(name="wd", bufs=2))
        pxs = ctx.enter_context(tc.tile_pool(name="xs", bufs=3))
        pq = ctx.enter_context(tc.tile_pool(name="qs", bufs=4))
        pex = ctx.enter_context(tc.tile_pool(name="ex", bufs=3))
        psc = ctx.enter_context(tc.tile_pool(name="sc", bufs=2))
        pps = ctx.enter_context(tc.tile_pool(name="ps", bufs=2, space="PSUM"))
        po = ctx.enter_context(tc.tile_pool(name="o", bufs=3))

        wd_tiles = {}

        def build_adapter(d):
            wd = pwd.tile([128, NB * OS], F16, tag="wd", name=f"wd{d}")
            for b in range(8):
                qt = pq.tile([128, OS], U16, tag="qt")
                nc.scalar.dma_start(qt[:], qst_d[d, b])
                for j in range(4):
                    kb = 4 * b + j
                    ex = pex.tile([128, OS], U16, tag="ex")
                    if j == 0:
                        nc.vector.tensor_scalar(
                            out=ex[:], in0=qt[:], scalar1=0xF, scalar2=None,
                            op0=mybir.AluOpType.bitwise_and,
                        )
                    elif j == 3:
                        nc.vector.tensor_scalar(
                            out=ex[:], in0=qt[:], scalar1=12, scalar2=None,
                            op0=mybir.AluOpType.logical_shift_right,
                        )
                    else:
                        nc.vector.tensor_scalar(
                            out=ex[:], in0=qt[:], scalar1=4 * j, scalar2=0xF,
                            op0=mybir.AluOpType.logical_shift_right,
                            op1=mybir.AluOpType.bitwise_and,
                        )
                    nc.vector.tensor_copy(wd[:, kb * OS : (kb + 1) * OS], ex[:])
            scb = psc.tile([128, OS], F32, tag="scb", name=f"scb{d}")
            nc.gpsimd.dma_start(scb[:], scb_d[d])
            return wd, scb

        # HAM warmup: the PE is idle for ~16us while startup DMAs land and
        # its clock is gated to 1.2 GHz until ~3.4us of sustained activity.
        # Burn ~5us of dummy matmuls into tile 0's future pb0 bank (start=True
        # on the real kb0 resets it) so real work begins at 2.4 GHz.
        warm_t = pex.tile([128, 512], F16, tag="warm")
        nc.gpsimd.memset(warm_t[:], 0.25)
        warm_ps = pps.tile([128, 512], F32, space="PSUM", tag="pb0", name="warm_ps")
        for _ in range(24):
            nc.tensor.matmul(
                warm_ps[:], lhsT=warm_t[:, 0:128], rhs=warm_t[:],
                start=True, stop=True,
            )
        warm_sb = pex.tile([1, 1], F32, tag="warmsb")
        nc.vector.tensor_copy(warm_sb[:], warm_ps[0:1, 0:1])
        nc.sync.dma_start(warm_d[:], warm_sb[:])

        # first x tile + first adapter's packed words land before the bulk
        # base-weight load so the PE can start at ~4us
        xs_first = pxs.tile([128, NB * 128], F16, tag="xs")
        nc.sync.dma_start(xs_first[:], xst_d[0])
        wd_tiles[tiles_spec[0][0][2]] = build_adapter(tiles_spec[0][0][2])

        # base weights: resident k-major f16, loaded in 8 dense chunks
        wbt = pwb.tile([128, NB * OS], F16, tag="wbt")
        chunk = NB * OS // 8
        for c in range(8):
            nc.gpsimd.dma_start(
                wbt[:, c * chunk : (c + 1) * chunk],
                wbt_d[:, c * chunk : (c + 1) * chunk],
            )

        for ti in range(n_tiles):
            segs = tiles_spec[ti]
            for _, _, d in segs:
                if d not in wd_tiles:
                    wd_tiles[d] = build_adapter(d)

            if ti == 0:
                xs = xs_first
            else:
                xs = pxs.tile([128, NB * 128], F16, tag="xs")
                nc.sync.dma_start(xs[:], xst_d[ti])

            pb0 = pps.tile([128, 512], F32, space="PSUM", tag="pb0")
            pb1 = pps.tile([128, 256], F32, space="PSUM", tag="pb1")
            pd0 = pps.tile([128, 512], F32, space="PSUM", tag="pd0")
            pd1 = pps.tile([128, 256], F32, space="PSUM", tag="pd1")

            # base phase first (no dependency on delta-weight extraction),
            # then delta phase. Two-bank alternation per phase keeps the PE
            # at full rate; 4-bank round-robin per kb measures ~20% slower
            # (PSUM-queue cycling micro-idles).
            for kb in range(NB):
                xk = xs[:, kb * 128 : (kb + 1) * 128]
                st = kb == 0
                sp = kb == NB - 1
                o0 = kb * OS
                nc.tensor.matmul(
                    pb0[:], lhsT=xk, rhs=wbt[:, o0 : o0 + 512], start=st, stop=sp
                )
                nc.tensor.matmul(
                    pb1[:], lhsT=xk, rhs=wbt[:, o0 + 512 : o0 + OS], start=st, stop=sp
                )
            for kb in range(NB):
                st = kb == 0
                sp = kb == NB - 1
                o0 = kb * OS
                for off, m, d in segs:
                    xm = xs[:, kb * 128 + off : kb * 128 + off + m]
                    nc.tensor.matmul(
                        pd0[off : off + m, :],
                        lhsT=xm,
                        rhs=wd_tiles[d][0][:, o0 : o0 + 512],
                        start=st, stop=sp,
                    )
                    nc.tensor.matmul(
                        pd1[off : off + m, :],
                        lhsT=xm,
                        rhs=wd_tiles[d][0][:, o0 + 512 : o0 + OS],
                        start=st, stop=sp,
                    )

            ot = po.tile([128, OS], F32, tag="ot")
            tmp = po.tile([128, OS], F32, tag="tmp")
            for off, m, d in segs:
                scb = wd_tiles[d][1]
                nc.vector.tensor_tensor(
                    out=tmp[off : off + m, 0:512],
                    in0=pd0[off : off + m, :],
                    in1=scb[off : off + m, 0:512],
                    op=mybir.AluOpType.mult,
                )
                nc.vector.tensor_tensor(
                    out=tmp[off : off + m, 512:OS],
                    in0=pd1[off : off + m, :],
                    in1=scb[off : off + m, 512:OS],
                    op=mybir.AluOpType.mult,
                )
            nc.vector.tensor_tensor(
                out=ot[:, 0:512], in0=tmp[:, 0:512], in1=pb0[:],
                op=mybir.AluOpType.add,
            )
            nc.vector.tensor_tensor(
                out=ot[:, 512:OS], in0=tmp[:, 512:OS], in1=pb1[:],
                op=mybir.AluOpType.add,
            )
            nc.scalar.dma_start(out_d[ti], ot[:])

    nc.compile()
    return nc


# ---------------------------------------------------------------------------
# Host wrapper
# ---------------------------------------------------------------------------
def _unpack_zeros(qz, o_count):
    # qz: [D, o_count//8, 1] int32; returns [D, o_count] float zeros
    o = np.arange(o_count)
    words = qz[:, o >> 3, 0].astype(np.int64)
    return ((words >> (4 * (o & 7))) & 0xF).astype(np.float32)


_prog_cache = {}


def kernel(**inputs):
    x = np.ascontiguousarray(np.asarray(inputs["x"], dtype=np.float32))
    w_base = np.asarray(inputs["w_base"], dtype=np.float32)
    bias = np.asarray(inputs["bias"], dtype=np.float32)
    qw_q = np.asarray(inputs["qweight_q"], dtype=np.int32)
    qw_k = np.asarray(inputs["qweight_k"], dtype=np.int32)
    qw_v = np.asarray(inputs["qweight_v"], dtype=np.int32)
    qz_q = np.asarray(inputs["qzeros_q"], dtype=np.int32)
    qz_k = np.asarray(inputs["qzeros_k"], dtype=np.int32)
    qz_v = np.asarray(inputs["qzeros_v"], dtype=np.int32)
    sc_q = np.asarray(inputs["scales_q"], dtype=np.float32)
    sc_k = np.asarray(inputs["scales_k"], dtype=np.float32)
    sc_v = np.asarray(inputs["scales_v"], dtype=np.float32)
    indices = np.asarray(inputs["indices"])

    tiles_spec, gather, origs = _schedule(indices)
    n_tiles = len(tiles_spec)
    t_pad = n_tiles * 128

    if tiles_spec not in _prog_cache:
        _prog_cache[tiles_spec] = _build_program(tiles_spec)
    nc = _prog_cache[tiles_spec]

    # sorted, k-permuted, f16, tile-major activations [n_tiles, 128, NB*128]
    xsT = np.ascontiguousarray(x[gather].T.astype(np.float16))  # [IN, t_pad]
    xx = xsT[KIDX].reshape(NB, 128, n_tiles, 128)
    xst = np.ascontiguousarray(
        xx.transpose(2, 1, 0, 3).reshape(n_tiles, 128, NB * 128)
    )

    z_q = _unpack_zeros(qz_q, Q)
    z_k = _unpack_zeros(qz_k, KV)
    z_v = _unpack_zeros(qz_v, KV)

    SQ, SK = Q // N_CORES, KV // N_CORES
    in_maps = []
    col_list = []
    for c in range(N_CORES):
        qs = slice(SQ * c, SQ * (c + 1))
        ks = slice(SK * c, SK * (c + 1))
        wb = np.concatenate(
            [w_base[qs], w_base[Q + SK * c : Q + SK * (c + 1)],
             w_base[Q + KV + SK * c : Q + KV + SK * (c + 1)]], axis=0
        )  # [OS, IN] f32
        ww = wb.astype(np.float16).T[KIDX]  # [NB*128, OS]
        wbt = np.ascontiguousarray(
            ww.reshape(NB, 128, OS).transpose(1, 0, 2).reshape(128, NB * OS)
        )
        qw = np.concatenate([qw_q[:, qs], qw_k[:, ks], qw_v[:, ks]], axis=1)
        qwu = np.ascontiguousarray(qw).view(np.uint16).reshape(D, OS, IN // 4)
        qst = np.ascontiguousarray(
            qwu.transpose(0, 2, 1).reshape(D, 8, 128, OS)
        )
        sc = np.concatenate(
            [sc_q[:, qs, 0], sc_k[:, ks, 0], sc_v[:, ks, 0]], axis=1
        ).astype(np.float32)  # [D, OS]
        scb = np.ascontiguousarray(
            np.broadcast_to(sc[:, None, :], (D, 128, OS))
        )
        in_maps.append({"xst": xst, "wbt": wbt, "qst": qst, "scb": scb})
        col_list.append(
            np.concatenate(
                [
                    np.arange(SQ * c, SQ * (c + 1)),
                    np.arange(Q + SK * c, Q + SK * (c + 1)),
                    np.arange(Q + KV + SK * c, Q + KV + SK * (c + 1)),
                ]
            )
        )

    import os

    trace = bool(int(os.environ.get("KERNEL_TRACE", "0")))
    res = run_bass_kernel_spmd(
        nc, in_maps, core_ids=list(range(N_CORES)), trace=trace
    )
    kernel._last_results = res

    out = np.zeros([T, OUT], np.float32)
    valid = origs >= 0
    vpos = np.nonzero(valid)[0]
    vtok = origs[valid]
    for c in range(N_CORES):
        r = np.asarray(res.results[c]["outD"]).reshape(t_pad, OS)
        out[vtok[:, None], col_list[c][None, :]] = r[vpos]

    # exact host-side rank-1 terms: bias + (-(z+1)*scale) * rowsum(x)
    sc_all = np.concatenate(
        [sc_q[:, :, 0], sc_k[:, :, 0], sc_v[:, :, 0]], axis=1
    ).astype(np.float32)  # [D, OUT]
    z_all = np.concatenate([z_q, z_k, z_v], axis=1)  # [D, OUT]
    znsc = -(z_all + 1.0) * sc_all  # [D, OUT]
    u_all = x.astype(np.float64).sum(axis=1).astype(np.float32)  # [T]
    out += bias[None, :]
    out += znsc[np.asarray(indices, dtype=np.int64)] * u_all[:, None]
    return out
